# revision 11
# baseline (speedup 1.0000x reference)
"""v5 candidate: 2x PE column-tiling (tile_position) variant.

Each round processes 2 images CONCURRENTLY as two 128x64 col-tiles:
  - tile j (j=0,1): lhsT [128, 64] shared weights, rhs = img j's SBUF tile,
    out -> psum partitions 64j..64j+63.
  - Contraction rows = (delta, c): delta in {0,1} is an x-shift of the input
    plane, pre-built on the HOST (free replication: just a second HBM copy).
    So one matmul covers 2 dx taps: 15 matmuls/tile/chunk instead of 25
    per 2 images -> ~1.67x less PE stream if tiles overlap.
"""

import numpy as np
import ml_dtypes

B, C, H, W = 32, 64, 112, 112
O, K, KS = 64, 8, 5
HO, WO = H - KS + 1, W - KS + 1
N_CORES = 8
ROUNDS = 2          # 2 rounds x 2 images = 4 images per core
NT = 15             # 5 dy * 3 dx-pairs
ROWS_PER_CHUNK = 4
N_CHUNKS = HO // ROWS_PER_CHUNK

_built = None


def _build_nc():
    import concourse.tile as tile
    import concourse.mybir as mybir
    from concourse import bacc

    nc = bacc.Bacc(None)
    x = nc.dram_tensor("x", [ROUNDS, 2, 128, H, W], mybir.dt.bfloat16,
                       kind="ExternalInput")
    w = nc.dram_tensor("w", [128, NT * 64], mybir.dt.bfloat16,
                       kind="ExternalInput")
    bt = nc.dram_tensor("b", [128, 1], mybir.dt.float32, kind="ExternalInput")
    y = nc.dram_tensor("y", [ROUNDS, 128, HO * WO], mybir.dt.float32,
                       kind="ExternalOutput")

    with tile.TileContext(nc) as tc:
        with (
            tc.tile_pool(name="wp", bufs=1) as wp,
            tc.tile_pool(name="xp", bufs=3) as xp,
            tc.tile_pool(name="op", bufs=4) as op,
            tc.tile_pool(name="bp", bufs=1) as bp,
            tc.tile_pool(name="ps", bufs=8, space="PSUM") as ps,
        ):
            # HAM warmup during DMA fill. Full-array matmuls: col-tiled
            # warmup MMs were observed NOT to engage the HAM clock gate;
            # the one mode-switch drain before the real MMs is cheap.
            warm = wp.tile([128, 64], mybir.dt.bfloat16, tag="warm")
            nc.vector.memset(warm[:], 0.0)
            wpsum = ps.tile([128, 64], mybir.dt.float32, tag="pt")
            for i in range(80):
                nc.tensor.matmul(wpsum[:64, :], warm[:], warm[:],
                                 start=True, stop=True)

            w3 = w.rearrange("p (t m) -> p t m", t=NT)
            wt = wp.tile([128, NT, 64], mybir.dt.bfloat16)

            xtiles = [xp.tile([128, H, W], mybir.dt.bfloat16, tag="xt",
                              name=f"xt{i}")
                      for i in range(ROUNDS * 2)]
            BAND = 16
            # both images' first rows, then weights, then interleaved bands
            for j in range(2):
                nc.sync.dma_start(xtiles[j][:, 0:8, :], x[0, j][:, 0:8, :])
            nc.sync.dma_start(wt[:, 0:1, :], w3[:, 0:1, :])
            for t0, t1 in ((1, 5), (5, 10), (10, NT)):
                nc.sync.dma_start(wt[:, t0:t1, :], w3[:, t0:t1, :])
            bias = bp.tile([128, 1], mybir.dt.float32)
            nc.sync.dma_start(bias[:], bt[:])
            for r in range(ROUNDS):
                if r > 0:
                    for j in range(2):
                        nc.sync.dma_start(xtiles[2 * r + j][:, 0:8, :],
                                          x[r, j][:, 0:8, :])
                for b0 in range(8, H, BAND):
                    b1 = min(b0 + BAND, H)
                    for j in range(2):
                        nc.sync.dma_start(xtiles[2 * r + j][:, b0:b1, :],
                                          x[r, j][:, b0:b1, :])

            for r in range(ROUNDS):
                for chunk in range(N_CHUNKS):
                    y0 = chunk * ROWS_PER_CHUNK
                    pt = ps.tile([128, ROWS_PER_CHUNK * WO], mybir.dt.float32,
                                 tag="pt")
                    for t in range(NT):
                        dy, dxp = divmod(t, 3)
                        for j in range(2):
                            for h in range(2):
                                p0 = 64 * j + 32 * h
                                nc.tensor.matmul(
                                    pt[p0 : p0 + 32, :],
                                    wt[:, t, 32 * h : 32 * h + 32],
                                    xtiles[2 * r + j][
                                        :, y0 + dy : y0 + dy + ROWS_PER_CHUNK,
                                        2 * dxp : 2 * dxp + WO],
                                    start=(t == 0),
                                    stop=(t == NT - 1),
                                    tile_position=(0, p0),
                                )
                    ot = op.tile([128, ROWS_PER_CHUNK * WO], mybir.dt.float32)
                    nc.vector.tensor_scalar_add(ot[:], pt[:], bias[:])
                    nc.sync.dma_start(
                        y[r][:, y0 * WO : (y0 + ROWS_PER_CHUNK) * WO], ot[:]
                    )
    nc.finalize()
    return nc


def _prep_inputs(X, weight, bias, sel):
    weight = np.asarray(weight)
    sel = np.asarray(sel)
    w64 = np.zeros((KS, KS, C, O), dtype=np.float32)
    wflat = weight.astype(np.float32)
    for o in range(O):
        for j in range(K):
            w64[:, :, int(sel[o, j]), o] += wflat[o, j]
    # lhsT rows = (delta*64 + c), tap15 = dy*3+dxp covers dx = 2*dxp+delta
    w2 = np.zeros((128, NT, O), dtype=np.float32)
    for dy in range(KS):
        for dxp in range(3):
            for d in range(2):
                dx = 2 * dxp + d
                if dx < KS:
                    w2[d * C : (d + 1) * C, dy * 3 + dxp, :] = w64[dy, dx]
    w_host = np.ascontiguousarray(w2.reshape(128, NT * O)).astype(
        ml_dtypes.bfloat16)

    b_host = np.tile(np.asarray(bias, dtype=np.float32), 2).reshape(128, 1)

    xb = np.asarray(X, dtype=np.float32).astype(ml_dtypes.bfloat16)
    # per image: [128, H, W]: rows 0-63 = plane, 64-127 = plane shifted
    # left by 1 in x (zero pad last col)
    xsh = np.zeros((B, 2, C, H, W), dtype=ml_dtypes.bfloat16)
    xsh[:, 0] = xb
    xsh[:, 1, :, :, : W - 1] = xb[:, :, :, 1:]
    xcores = xsh.reshape(N_CORES, ROUNDS, 2, 2 * C, H, W)

    in_maps = [
        {"x": np.ascontiguousarray(xcores[i]), "w": w_host, "b": b_host}
        for i in range(N_CORES)
    ]
    return in_maps


def _postprocess(results):
    outs = []
    for r in results:
        outs.append(r["y"].reshape(ROUNDS * 2, O, HO, WO))
    return np.concatenate(outs, axis=0).astype(np.float32)


def kernel(X, weight, bias, sel):
    global _built
    from concourse.bass_utils import run_bass_kernel_spmd

    assert X.shape == (B, C, H, W), X.shape
    if _built is None:
        _built = _build_nc()
    in_maps = _prep_inputs(X, weight, bias, sel)
    res = run_bass_kernel_spmd(
        _built, in_maps, core_ids=list(range(N_CORES)), trace=False
    )
    return _postprocess(res.results)
